# revision 38
# baseline (speedup 1.0000x reference)
"""Distributed causal RoPE attention for Trainium2 (8 NeuronCores).

Problem: nn_CausalRpeAttn — B=2, S=2048, D=1024, H=16, Dh=64, fp32.

Sharding (data + head parallel): core c handles batch c//4 and heads
4*(c%4) .. 4*(c%4)+3 (a 256-wide feature slice). Wq/Wk/Wv are split
column-wise (by output head group), Wo row-wise. Each core writes its
full [1024, 2048] (transposed, bf16) partial output projection (with
bo/4 pre-added); the host unshards by summing the 4 partials per batch
and transposing back. Attention itself is fully independent per
(batch, head), so the only cross-core combination is that final sum.

Schedule: everything is slab-pipelined over 512-position s-slabs so the
PE never sits at a phase boundary:
  proj(0) proj(1) attn(0) proj(2) attn(1) wo(0) proj(3) attn(2) wo(1)
  attn(3) wo(2) wo(3)
proj(st) = QKV projections + RoPE + V transpose for slab st; attn(qt)
consumes k/v slabs 0..qt, so attn(qt) is issued right after proj(qt+1)
and its RoPE/exp/normalize runs on Scalar/Vector/GpSimd while the PE
streams the next slab's matmuls.

On-device layout notes:
 - Feature-major ("transposed", [feat, seq]) so all matmuls have moving
   dim 512. QKV projections in bf16; q/k after RoPE bf16.
 - RoPE's rotate-half runs on the PE as a signed permutation matmul
   (host-built [128,128] matrix, sign folded in), so no SBUF->SBUF DMAs
   sit on the serial Sync descriptor queue; the rest is a Scalar
   bias-add, two DVE muls against bf16 tables, and a GpSimd add.
 - Scores transposed sT[k_pos, q] per head, two heads quadrant-packed on
   the PE (Dh=64 contraction, lhsT base partitions 0/64); diagonal tiles
   trim the matmul+exp to the causal width. Softmax skips the max
   subtraction (scores/8 are O(5)); exp on Scalar straight out of PSUM
   with the 1/8 scale folded in, bf16 probs; diagonal masked by gpsimd
   affine_select.
 - v carries an appended ones-row so PV (lhsT=[128,65]) yields weighted
   sum + denominator in one PSUM tile; denominators staged to SBUF (the
   custom-DVE reciprocal cannot read PSUM), 1/denom via single-pass
   reciprocal_approx_fast (18-bit), broadcast on GpSimd, normalized by
   two DVE muls into the fp32r Wo input.
 - Wo partials accumulate into one [128,8,512] bf16 staging tile per
   q tile and leave in a single 1MB DMA (half-size, bf16); host sums
   the 4 partials per batch in fp32.
"""

import os
import ml_dtypes
import numpy as np

B, S, D, H, DH = 2, 2048, 1024, 16, 64
N_CORES = 8
FPC = 256  # features per core (4 heads)
QT = 512
NQT = S // QT  # 4
NST = S // 512  # 4 s-slabs

_cache = {}
last_run_info = {}


def _build():
    import concourse.bass as bass
    import concourse.mybir as mybir
    import concourse.tile as tile
    from concourse import bacc
    from concourse.bass import broadcast_tensor_aps
    from concourse.masks import make_identity

    F32 = mybir.dt.float32
    F32R = mybir.dt.float32r
    BF16 = mybir.dt.bfloat16
    AOP = mybir.AluOpType
    EXP = mybir.ActivationFunctionType.Exp
    IDENT = mybir.ActivationFunctionType.Identity

    nc = bacc.Bacc("TRN2", target_bir_lowering=False, debug=False,
                   num_devices=N_CORES)

    qkvT_e = nc.dram_tensor("qkvT", [128, NST, 8, 512], BF16, kind="ExternalInput").ap()
    wq_e = nc.dram_tensor("wq", [128, 8, FPC], BF16, kind="ExternalInput").ap()
    wk_e = nc.dram_tensor("wk", [128, 8, FPC], BF16, kind="ExternalInput").ap()
    wv_e = nc.dram_tensor("wv", [128, 8, FPC], BF16, kind="ExternalInput").ap()
    wo_e = nc.dram_tensor("wo", [FPC, D], F32R, kind="ExternalInput").ap()
    bq_e = nc.dram_tensor("bq", [FPC], F32, kind="ExternalInput").ap()
    bk_e = nc.dram_tensor("bk", [FPC], F32, kind="ExternalInput").ap()
    bv_e = nc.dram_tensor("bv", [FPC], F32, kind="ExternalInput").ap()
    bo_e = nc.dram_tensor("bo", [D], F32, kind="ExternalInput").ap()
    cos2_e = nc.dram_tensor("cos2", [128, S], BF16, kind="ExternalInput").ap()
    sinx_e = nc.dram_tensor("sinx", [128, S], BF16, kind="ExternalInput").ap()
    perm_e = nc.dram_tensor("perm", [128, 128], BF16, kind="ExternalInput").ap()
    out_e = nc.dram_tensor("out", [128, NQT, 8, 512], BF16, kind="ExternalOutput").ap()

    from contextlib import ExitStack
    with tile.TileContext(nc) as tc:
        with ExitStack() as ctx:
            ep = ctx.enter_context
            consts = ep(tc.tile_pool(name="consts", bufs=1))
            xin_pool = ep(tc.tile_pool(name="xin", bufs=1))
            rope_pool = ep(tc.tile_pool(name="rope", bufs=16))
            tmp_pool = ep(tc.tile_pool(name="tmp", bufs=2))
            qb_pool = ep(tc.tile_pool(name="qb", bufs=3))
            vtmp_pool = ep(tc.tile_pool(name="vtmp", bufs=2))
            vsb_pool = ep(tc.tile_pool(name="vsb", bufs=4))
            probs_pool = ep(tc.tile_pool(name="probs", bufs=4))
            woin_pool = ep(tc.tile_pool(name="woin", bufs=8))
            rec_pool = ep(tc.tile_pool(name="rec", bufs=4))
            pvs_pool = ep(tc.tile_pool(name="pvs", bufs=2))
            rb_pool = ep(tc.tile_pool(name="rb", bufs=2))
            osb_pool = ep(tc.tile_pool(name="osb", bufs=2))
            # PSUM: scores 2x[128,1024] (4 banks) + pv 2x[65,512] (2) +
            # proj/rope-perm/v-transpose/wo 2x[128,512] (2) = 8 banks.
            ps_pool = ep(tc.tile_pool(name="ps", bufs=2, space="PSUM"))
            pv_pool = ep(tc.tile_pool(name="pv", bufs=2, space="PSUM"))
            big_pool = ep(tc.tile_pool(name="big", bufs=2, space="PSUM"))

            # ---- constants + inputs, ordered so compute starts early.
            # x / weights / out are host-packed so every DMA moves >=4KB
            # contiguous per partition (1KB rows run ~3x slower) ----
            # wq + x slab 0 first so the first projections can start
            wq_sb = consts.tile([128, 8, FPC], BF16, tag="wq")
            nc.sync.dma_start(out=wq_sb[:], in_=wq_e)
            x_all = xin_pool.tile([128, NST, 8, 512], BF16, tag="x")
            nc.sync.dma_start(out=x_all[:, 0, 0:4], in_=qkvT_e[:, 0, 0:4])
            nc.sync.dma_start(out=x_all[:, 0, 4:8], in_=qkvT_e[:, 0, 4:8])

            b_sbs = []
            for name, be in (("bq", bq_e), ("bk", bk_e), ("bv", bv_e)):
                t = consts.tile([128, 2], F32, tag=name)
                nc.sync.dma_start(out=t[:], in_=be.rearrange("(t p) -> p t", p=128))
                b_sbs.append(t)
            bq_sb, bk_sb, bv_sb = b_sbs
            perm_sb = consts.tile([128, 128], BF16, tag="perm")
            nc.sync.dma_start(out=perm_sb[:], in_=perm_e)
            cos2_sb = consts.tile([128, S], BF16, tag="cos2")
            nc.sync.dma_start(out=cos2_sb[:], in_=cos2_e)
            sinx_sb = consts.tile([128, S], BF16, tag="sinx")
            nc.sync.dma_start(out=sinx_sb[:], in_=sinx_e)
            wk_sb = consts.tile([128, 8, FPC], BF16, tag="wk")
            wv_sb = consts.tile([128, 8, FPC], BF16, tag="wv")
            for t, we in ((wk_sb, wk_e), (wv_sb, wv_e)):
                nc.sync.dma_start(out=t[:], in_=we)
            # rest of x, slab-ordered so proj(st) can start promptly
            for st in range(1, NST):
                nc.sync.dma_start(out=x_all[:, st], in_=qkvT_e[:, st])
            wo_sb = consts.tile([128, 2, D], F32R, tag="wo")
            nc.sync.dma_start(out=wo_sb[:],
                              in_=wo_e.rearrange("(pt p) f -> p pt f", p=128))
            bo_sb = consts.tile([128, 8], F32, tag="bo")
            nc.sync.dma_start(out=bo_sb[:], in_=bo_e.rearrange("(t p) -> p t", p=128))
            identb = consts.tile([128, 128], BF16, tag="identb")
            make_identity(nc, identb[:])
            maskT = consts.tile([128, 128], BF16, tag="maskT")
            nc.gpsimd.memset(maskT[:], 0.0)
            nc.gpsimd.affine_select(
                out=maskT[:], in_=maskT[:],
                pattern=[[-1, 128]], compare_op=AOP.is_ge,
                fill=-10000.0, base=0, channel_multiplier=1)

            # rope targets: per (pt, slab) tiles so slab st+1 writes never
            # serialize against attention reads of earlier slabs
            qrot = [[rope_pool.tile([128, 512], BF16, tag="rope",
                                    name=f"qrot{i}_{s}") for s in range(NST)]
                    for i in range(2)]
            krot = [[rope_pool.tile([128, 512], BF16, tag="rope",
                                    name=f"krot{i}_{s}") for s in range(NST)]
                    for i in range(2)]
            # v with ones row, per slab: [s_tile part, 4 s-tiles, 4 heads, 65]
            v_sb = [vsb_pool.tile([128, 4, 4, DH + 1], BF16, tag="v",
                                  name=f"v{s}") for s in range(NST)]
            for s in range(NST):
                nc.vector.memset(
                    v_sb[s][:].rearrange("p a b c -> p (a b c)"), 1.0)

            def rope_finish(qb, dst, ss):
                # signed rotate-half via PE permutation matmul
                qbs_ps = ps_pool.tile([128, 512], F32, tag="ps", name="qbs")
                nc.tensor.matmul(qbs_ps[:], perm_sb[:], qb[:],
                                 start=True, stop=True)
                nc.vector.tensor_mul(out=dst[:], in0=qb[:],
                                     in1=cos2_sb[:, ss])
                tmp = tmp_pool.tile([128, 512], BF16, tag="tmp")
                nc.vector.tensor_mul(out=tmp[:], in0=qbs_ps[:],
                                     in1=sinx_sb[:, ss])
                nc.vector.tensor_add(out=dst[:], in0=dst[:], in1=tmp[:])

            # ---- slab projections + RoPE + v transpose (generator:
            # yields 12x per slab so attention can interleave) ----
            def proj_slab_gen(st):
                ss = slice(st * 512, (st + 1) * 512)
                pending = []  # deferred rope finishes, run after next block

                def run_pending():
                    while pending:
                        rope_finish(*pending.pop())

                for proj in range(3):
                    w_sb = (wq_sb, wk_sb, wv_sb)[proj]
                    b_sb = b_sbs[proj]
                    for pt in range(2):
                        ps = ps_pool.tile([128, 512], F32, tag="ps")
                        for kt in range(8):
                            nc.tensor.matmul(
                                ps[:], w_sb[:, kt, pt * 128:(pt + 1) * 128],
                                x_all[:, st, kt, :],
                                start=(kt == 0), stop=(kt == 7))
                        if proj < 2:
                            qb = qb_pool.tile([128, 512], BF16, tag="qb")
                            nc.scalar.activation(
                                out=qb[:], in_=ps[:], func=IDENT,
                                bias=b_sb[:, pt:pt + 1])
                            run_pending()
                            pending.append((qb, (qrot, krot)[proj][pt][st], ss))
                        else:
                            vt = vtmp_pool.tile([128, 512], BF16, tag="vt")
                            nc.scalar.activation(
                                out=vt[:], in_=ps[:], func=IDENT,
                                bias=bv_sb[:, pt:pt + 1])
                            run_pending()
                            ps4 = ps_pool.tile([128, 512], BF16, tag="ps",
                                               name="ps4")
                            for j in range(4):
                                nc.tensor.transpose(
                                    ps4[:, j * 128:(j + 1) * 128],
                                    vt[:, j * 128:(j + 1) * 128],
                                    identb[:])
                            nc.vector.tensor_copy(
                                out=v_sb[st][:, :, 2 * pt:2 * pt + 2, 0:DH],
                                in_=ps4[:].rearrange(
                                    "p (j h d) -> p j h d", j=4, h=2))
                        if pt == 1:
                            yield
                run_pending()

            # ---- attention (qt) + per-qt Wo partial ----
            norm_state = {}
            woin = [[woin_pool.tile([128, 512], F32R, tag="woin",
                                    name=f"woin{i}_{q}") for q in range(NQT)]
                    for i in range(2)]

            def scores(kt, qt, pair, qsl):
                kst, ko = kt // 4, (kt % 4) * 128
                ksl = slice(ko, ko + 128)
                ps_s = big_pool.tile([128, 1024], F32, tag="big",
                                     name="ps_s")
                off = kt * 128 - qt * 512
                o = max(0, off)
                kr = krot[pair][kst]
                qr = qrot[pair][qt]
                diag = off >= 0
                nc.tensor.matmul(
                    ps_s[:, o:512], kr[0:64, ksl],
                    qr[0:64, o:512], start=True, stop=not diag)
                nc.tensor.matmul(
                    ps_s[:, 512 + o:1024], kr[64:128, ksl],
                    qr[64:128, o:512], start=True, stop=not diag)
                if diag:
                    # causal mask folded into the PE accumulation: adds
                    # -1e4 to the strict-upper triangle of the diagonal
                    # 128x128 block so exp underflows to zero
                    nc.tensor.matmul(
                        ps_s[:, o:o + 128], maskT[:], identb[:],
                        start=False, stop=True, skip_group_check=True)
                    nc.tensor.matmul(
                        ps_s[:, 512 + o:512 + o + 128], maskT[:], identb[:],
                        start=False, stop=True, skip_group_check=True)
                pr = probs_pool.tile([128, 1024], BF16, tag="pr", name="pr")
                psv = ps_s[:].rearrange("p (h q) -> p h q", h=2)
                prv = pr[:].rearrange("p (h q) -> p h q", h=2)
                if off <= 0:
                    nc.scalar.activation(out=pr[:], in_=ps_s[:],
                                         func=EXP, scale=0.125)
                else:
                    nc.scalar.activation(out=prv[:, :, off:512],
                                         in_=psv[:, :, off:512],
                                         func=EXP, scale=0.125)
                return pr

            def pv(kt, pr, pv_a, pv_b, h0, nkt, qt):
                off = max(0, kt * 128 - qt * 512)
                vs = v_sb[kt // 4]
                nc.tensor.matmul(
                    pv_a[:, off:512], vs[:, kt % 4, h0, :],
                    pr[:, off:512],
                    start=(kt == 0), stop=(kt == nkt - 1))
                nc.tensor.matmul(
                    pv_b[:, off:512], vs[:, kt % 4, h0 + 1, :],
                    pr[:, 512 + off:1024],
                    start=(kt == 0), stop=(kt == nkt - 1))

            def attn_gen(qt):
                qsl = slice(qt * 512, (qt + 1) * 512)
                nkt = 4 * qt + 4
                deferred = [None]

                def drain(pv_a, pv_b, pair):
                    # tail drain (last pair of the kernel): denominator rows
                    # first so recip + broadcast overlap the bulk pv copies;
                    # mid-kernel: bulk copies first so the PSUM banks free
                    # for the next pair as soon as possible
                    tail = qt == NQT - 1 and pair == 1
                    den = rec_pool.tile([1, 1024], F32, tag="rec",
                                        name="den")
                    pvs = pvs_pool.tile([DH + 1, 1024], F32, tag="pvs")

                    def den_part():
                        nc.vector.tensor_copy(out=den[0:1, 0:512],
                                              in_=pv_a[DH:DH + 1, :])
                        nc.vector.tensor_copy(out=den[0:1, 512:1024],
                                              in_=pv_b[DH:DH + 1, :])
                        nc.vector.reciprocal_approx_fast(
                            out=rec[0:1, :], in_=den[0:1, :])

                    def pvs_part():
                        nc.vector.tensor_copy(out=pvs[:, 0:512], in_=pv_a[:])
                        nc.vector.tensor_copy(out=pvs[:, 512:1024],
                                              in_=pv_b[:])

                    rec = rec_pool.tile([1, 1024], F32, tag="rec")
                    if tail:
                        den_part()
                        pvs_part()
                    else:
                        pvs_part()
                        den_part()
                    norm_state[(qt, pair)] = (pvs, rec)

                for pair in range(2):
                    pv_a = pv_pool.tile([DH + 1, 512], F32, tag="pv",
                                        name="pv_a")
                    pv_b = pv_pool.tile([DH + 1, 512], F32, tag="pv",
                                        name="pv_b")
                    h0 = 2 * pair

                    # software pipeline: scores run two kt ahead of pv;
                    # the previous pair's drain is emitted after this
                    # pair's first scores so the PE/Scalar have runway
                    prs = [scores(0, qt, pair, qsl), scores(1, qt, pair, qsl)]
                    if deferred[0] is not None:
                        deferred[0]()
                        deferred[0] = None
                    yield
                    for kt in range(2, nkt):
                        prs.append(scores(kt, qt, pair, qsl))
                        pv(kt - 2, prs.pop(0), pv_a, pv_b, h0, nkt, qt)
                        yield
                    pv(nkt - 2, prs.pop(0), pv_a, pv_b, h0, nkt, qt)
                    pv(nkt - 1, prs.pop(0), pv_a, pv_b, h0, nkt, qt)
                    deferred[0] = (lambda a=pv_a, b=pv_b, p=pair:
                                   drain(a, b, p))
                    yield
                deferred[0]()

            def wo_norm(qt, pair):
                pvs, rec = norm_state.pop((qt, pair))
                rb = rb_pool.tile([64, 1024], F32, tag="rb")
                nc.gpsimd.partition_broadcast(rb[:], rec[0:1, :])
                wt = woin[pair][qt]
                nc.vector.tensor_mul(out=wt[0:64, :],
                                     in0=pvs[0:DH, 0:512],
                                     in1=rb[:, 0:512])
                nc.vector.tensor_mul(out=wt[64:128, :],
                                     in0=pvs[0:DH, 512:1024],
                                     in1=rb[:, 512:1024])

            def wo_gen(qt, split_out=False):
                for pair in range(2):
                    wo_norm(qt, pair)
                ob = osb_pool.tile([128, 8, QT], BF16, tag="ot")
                for dm in range(8):
                    ps_o = ps_pool.tile([128, 512], F32, tag="ps",
                                        name="ps_o")
                    for pt in range(2):
                        nc.tensor.matmul(
                            ps_o[:], wo_sb[:, pt, dm * 128:(dm + 1) * 128],
                            woin[pt][qt][:], start=(pt == 0), stop=(pt == 1))
                    nc.vector.tensor_scalar_add(out=ob[:, dm, :],
                                                in0=ps_o[:],
                                                scalar1=bo_sb[:, dm:dm + 1])
                    if split_out and dm % 4 == 3:
                        nc.sync.dma_start(out=out_e[:, qt, dm - 3:dm + 1, :],
                                          in_=ob[:, dm - 3:dm + 1, :])
                    if dm % 2 == 1:
                        yield

                if not split_out:
                    nc.sync.dma_start(out=out_e[:, qt], in_=ob[:])

            def wo_tail(qt):
                # last q tile: pair0's pt=0 partials run during pair1's
                # drain gap, parked in the (now free) scores PSUM banks
                wo_norm(qt, 0)
                bigs = [big_pool.tile([128, 1024], F32, tag="big",
                                      name=f"wob{i}") for i in range(2)]
                for dm in range(4):
                    nc.tensor.matmul(
                        bigs[dm // 2][:, (dm % 2) * 512:(dm % 2) * 512 + 512],
                        wo_sb[:, 0, dm * 128:(dm + 1) * 128],
                        woin[0][qt][:], start=True, stop=False)
                wo_norm(qt, 1)
                ob = osb_pool.tile([128, 8, QT], BF16, tag="ot")
                for dm in range(8):
                    if dm < 4:
                        ps_o = bigs[dm // 2][:, (dm % 2) * 512:
                                             (dm % 2) * 512 + 512]
                        nc.tensor.matmul(
                            ps_o, wo_sb[:, 1, dm * 128:(dm + 1) * 128],
                            woin[1][qt][:], start=False, stop=True)
                    else:
                        pso_t = ps_pool.tile([128, 512], F32, tag="ps",
                                             name="ps_o")
                        ps_o = pso_t[:]
                        for pt in range(2):
                            nc.tensor.matmul(
                                ps_o, wo_sb[:, pt, dm * 128:(dm + 1) * 128],
                                woin[pt][qt][:], start=(pt == 0),
                                stop=(pt == 1))
                    nc.vector.tensor_scalar_add(out=ob[:, dm, :],
                                                in0=ps_o,
                                                scalar1=bo_sb[:, dm:dm + 1])
                    if dm % 4 == 3:
                        nc.sync.dma_start(out=out_e[:, qt, dm - 3:dm + 1, :],
                                          in_=ob[:, dm - 3:dm + 1, :])

            # ---- schedule: interleave attention kt-steps with the next
            # slab's projection blocks and the previous q tile's Wo blocks
            # so the PE and Scalar engines both stream continuously ----
            def interleave(main, n_main, fillers, n_fill):
                from itertools import chain
                fit = chain(*fillers)
                done_f = 0
                i = 0
                for _ in main:
                    i += 1
                    target = (i * n_fill) // n_main
                    while done_f < target:
                        try:
                            next(fit)
                        except StopIteration:
                            done_f = n_fill
                            break
                        done_f += 1
                for _ in fit:
                    pass

            if os.environ.get("KERNEL_FLAT_SCHED"):
                for g in (proj_slab_gen(0), proj_slab_gen(1), attn_gen(0),
                          proj_slab_gen(2), attn_gen(1), wo_gen(0),
                          proj_slab_gen(3), attn_gen(2), wo_gen(1),
                          attn_gen(3), wo_gen(2), wo_gen(3, split_out=True)):
                    for _ in g:
                        pass
            else:
                for _ in proj_slab_gen(0):
                    pass
                plan = [
                    (attn_gen(0), 8, [proj_slab_gen(1)], 3),
                    (attn_gen(1), 16, [proj_slab_gen(2), wo_gen(0)], 7),
                    (attn_gen(2), 24, [proj_slab_gen(3)], 3),
                    (attn_gen(3), 32, [wo_gen(1), wo_gen(2)], 8),
                ]
                for main, nm, fills, nf in plan:
                    interleave(main, nm, fills, nf)
                wo_tail(3)

    nc.compile()
    return nc


def kernel(qkv, cos, sin, Wq, bq, Wk, bk, Wv, bv, Wo, bo):
    from concourse.bass_utils import run_bass_kernel_spmd

    qkv = np.asarray(qkv, dtype=np.float32)
    cos = np.asarray(cos, dtype=np.float32)
    sin = np.asarray(sin, dtype=np.float32)
    Wq, bq = np.asarray(Wq, np.float32), np.asarray(bq, np.float32)
    Wk, bk = np.asarray(Wk, np.float32), np.asarray(bk, np.float32)
    Wv, bv = np.asarray(Wv, np.float32), np.asarray(bv, np.float32)
    Wo, bo = np.asarray(Wo, np.float32), np.asarray(bo, np.float32)

    if "nc" not in _cache:
        _cache["nc"] = _build()
    nc = _cache["nc"]

    bf = ml_dtypes.bfloat16
    cos2 = np.ascontiguousarray(np.tile(cos.T, (2, 1)).astype(bf))  # [128, S]
    sinx = np.ascontiguousarray(np.tile(sin.T, (2, 1)).astype(bf))

    # signed rotate-half permutation: out[m] = -q[m+32] (m%64<32),
    # +q[m-32] (m%64>=32); used as matmul lhsT.
    perm = np.zeros((128, 128), np.float32)
    for m in range(128):
        if (m % 64) < 32:
            perm[m + 32, m] = -1.0
        else:
            perm[m - 32, m] = 1.0
    perm = np.ascontiguousarray(perm.astype(bf))

    bo4 = np.ascontiguousarray(bo * 0.25)
    in_maps = []
    for c in range(N_CORES):
        b, g = c // 4, c % 4
        hsl = slice(g * FPC, (g + 1) * FPC)
        in_maps.append({
            "qkvT": np.ascontiguousarray(
                qkv[b].T.astype(bf).reshape(8, 128, NST, 512)
                .transpose(1, 2, 0, 3)),
            "wq": np.ascontiguousarray(
                Wq[hsl, :].T.astype(bf).reshape(8, 128, FPC)
                .transpose(1, 0, 2)),
            "wk": np.ascontiguousarray(
                Wk[hsl, :].T.astype(bf).reshape(8, 128, FPC)
                .transpose(1, 0, 2)),
            "wv": np.ascontiguousarray(
                Wv[hsl, :].T.astype(bf).reshape(8, 128, FPC)
                .transpose(1, 0, 2)),
            "wo": np.ascontiguousarray(Wo[:, hsl].T),
            "bq": np.ascontiguousarray(bq[hsl]),
            "bk": np.ascontiguousarray(bk[hsl]),
            "bv": np.ascontiguousarray(bv[hsl]),
            "bo": bo4,
            "cos2": cos2,
            "sinx": sinx,
            "perm": perm,
        })

    trace = bool(os.environ.get("KERNEL_TRACE"))
    res = run_bass_kernel_spmd(nc, in_maps, list(range(N_CORES)), trace=trace)
    last_run_info["exec_time_ns"] = res.exec_time_ns
    last_run_info["results"] = res

    out = np.empty((B, S, D), dtype=np.float32)
    for b in range(B):
        acc = (res.results[4 * b]["out"].astype(np.float32)
               + res.results[4 * b + 1]["out"].astype(np.float32)
               + res.results[4 * b + 2]["out"].astype(np.float32)
               + res.results[4 * b + 3]["out"].astype(np.float32))
        # [128 p, 4 qt, 8 dm, 512 q'] -> oT [dm*128+p, qt*512+q']
        oT = acc.transpose(2, 0, 1, 3).reshape(D, S)
        out[b] = oT.T
    return out
